# revision 8
# baseline (speedup 1.0000x reference)
"""SINDy autoencoder forward pass on 8 Trainium2 NeuronCores.

Data-parallel: batch (16384) sharded 8 ways. Everything is computed in
feature-major ("transposed") layout on device so that matmuls contract over
the SBUF partition dimension:

    z.T[f, b] = sum_k W.T[k, f] * x.T[k, b]   (lhsT = W.T, rhs = x.T)

The host pre-transposes x / x_dot (concatenating the treatment row so the
2049-wide first layer is a plain K-chunk loop), pre-transposes all weights,
and packs every small weight/bias/selector into one [128, 360] tensor that is
DMA'd once.

Matmul operands are stored as float32r (TRN2's fast fp32 PE mode, ~tf32
accuracy, full PE rate at free-dim >= 256); fp32 bits pass through DMA
unchanged and the PE rounds internally. The z / x_hat output paths and the
loss reductions stay in plain fp32.

Scalar losses are accumulated on device into per-(batch-tile, chunk) partial
sums (fused square+row-reduce on the ACT/DVE engines) and reduced on host.
l1 = mean(|coefficients|) is computed directly on host (21 elements).
"""

import numpy as np

import sys

if "/opt/trn_rl_repo" not in sys.path:
    sys.path.insert(0, "/opt/trn_rl_repo")

from contextlib import ExitStack

import concourse.bacc as bacc
import concourse.mybir as mybir
from concourse import tile
from concourse.bass_utils import run_bass_kernel_spmd

F32 = mybir.dt.float32
F32R = mybir.dt.float32r
AF = mybir.ActivationFunctionType
OP = mybir.AluOpType

B = 16384          # full batch
D = 2048           # input dim
NC_CORES = 8
BS = B // NC_CORES  # batch shard per core = 2048
NT = 256            # batch-tile (free dim of matmuls)
NBT = BS // NT      # 8 batch tiles per core
KC = D // 128       # 16 K-chunks of the input dim
MC = D // 128       # 16 output chunks of the decoder last layer
PBT = 40            # partials columns per batch tile
# partials col layout per batch tile: [0:16) recon, [16:32) sindy_x,
# col 32 = loss_po (row 0), col 33 = loss_tr (row 0), col 34 = sindy_z (rows 0:3)

# --- packed small-weight column layout (partition dim x free dim [128, 360]) --
SW_W2 = (128, 0, 64)      # eW2.T  [128, 64]
SW_W3 = (64, 64, 32)      # eW3.T  [64, 32]
SW_W4 = (32, 96, 3)       # eW4.T  [32, 3]
SW_V1 = (3, 99, 32)       # dW1.T  [3, 32]
SW_V2 = (32, 131, 64)     # dW2.T  [32, 64]
SW_V3 = (64, 195, 128)    # dW3.T  [64, 128]
SW_DB4 = (128, 323, 16)   # db4 as [128, 16] (col m = db4[m*128:(m+1)*128])
SW_EB1 = (128, 339, 1)
SW_EB2 = (64, 340, 1)
SW_EB3 = (32, 341, 1)
SW_EB4 = (3, 342, 1)
SW_DB1 = (32, 343, 1)
SW_DB2 = (64, 344, 1)
SW_DB3 = (128, 345, 1)
SW_SELH = (3, 346, 3)     # replicates z row 0 (s) onto 3 partitions
SW_SELL = (3, 349, 1)     # selects z row 2 (logits) onto partition 0
SW_CZ = (3, 350, 3)       # SINDy coeff rows applied to [s,d,t]
SW_CP = (3, 353, 3)       # ... applied to [s^2, sd, st]
SW_CQ = (3, 356, 3)       # ... applied to [s^3, s^2 d, s^2 t] (s^3 row zeroed)
SW_C0 = (3, 359, 1)       # constant theta row contribution (bias)
SW_COLS = 360

_BUILt = None


def build():
    """Build the (single-program, 8-core SPMD) Bass module once."""
    global _BUILt
    if _BUILt is not None:
        return _BUILt

    nc = bacc.Bacc("TRN2", target_bir_lowering=False, debug=False,
                   num_devices=NC_CORES)

    xt_d = nc.dram_tensor("xt", [D + 1, BS], F32R, kind="ExternalInput")
    xdt_d = nc.dram_tensor("xdt", [D + 1, BS], F32R, kind="ExternalInput")
    size_d = nc.dram_tensor("size", [1, BS], F32, kind="ExternalInput")
    w1_d = nc.dram_tensor("w1T", [D + 1, 128], F32R, kind="ExternalInput")
    v4_d = nc.dram_tensor("v4T", [128, D], F32R, kind="ExternalInput")
    sw_d = nc.dram_tensor("smallw", [128, SW_COLS], F32R, kind="ExternalInput")

    z_d = nc.dram_tensor("zT", [3, BS], F32, kind="ExternalOutput")
    xh_d = nc.dram_tensor("xhatT", [D, BS], F32, kind="ExternalOutput")
    pr_d = nc.dram_tensor("partials", [128, NBT * PBT], F32,
                          kind="ExternalOutput")

    def f(ap):
        """Raw-bits view of an fp32r AP for ACT/DVE (non-PE) use."""
        return ap.bitcast(F32)

    with tile.TileContext(nc) as tc, ExitStack() as ctx:
        wp = ctx.enter_context(tc.tile_pool(name="w", bufs=1))
        xp = ctx.enter_context(tc.tile_pool(name="x", bufs=2))
        ap_ = ctx.enter_context(tc.tile_pool(name="act", bufs=2))
        sp = ctx.enter_context(tc.tile_pool(name="strm", bufs=4))
        cp_ = ctx.enter_context(tc.tile_pool(name="accs", bufs=2))
        pp = ctx.enter_context(
            tc.tile_pool(name="ps", bufs=8, space="PSUM"))

        # ---- static weights, loaded once -----------------------------------
        w1_sb = wp.tile([128, KC * 128], F32R)
        for k in range(KC):
            nc.sync.dma_start(out=w1_sb[:, k * 128:(k + 1) * 128],
                              in_=w1_d[k * 128:(k + 1) * 128, :])
        w1r_sb = wp.tile([1, 128], F32R)
        nc.sync.dma_start(out=w1r_sb[:], in_=w1_d[D:D + 1, :])
        v4_sb = wp.tile([128, D], F32R)
        nc.sync.dma_start(out=v4_sb[:], in_=v4_d[:, :])
        sw = wp.tile([128, SW_COLS], F32R)
        nc.sync.dma_start(out=sw[:], in_=sw_d[:, :])

        def swslice(spec):
            p, c0, w = spec
            return sw[0:p, c0:c0 + w]

        w2 = swslice(SW_W2)
        w3 = swslice(SW_W3)
        w4 = swslice(SW_W4)
        v1 = swslice(SW_V1)
        v2 = swslice(SW_V2)
        v3 = swslice(SW_V3)
        db4m = f(swslice(SW_DB4))
        eb1 = f(swslice(SW_EB1))
        eb2 = f(swslice(SW_EB2))
        eb3 = f(swslice(SW_EB3))
        eb4 = f(swslice(SW_EB4))
        db1 = f(swslice(SW_DB1))
        db2 = f(swslice(SW_DB2))
        db3 = f(swslice(SW_DB3))
        selh = swslice(SW_SELH)
        sell = swslice(SW_SELL)
        cz = swslice(SW_CZ)
        cpm = swslice(SW_CP)
        cq = swslice(SW_CQ)
        c0 = f(swslice(SW_C0))

        for bt in range(NBT):
            b0 = bt * NT
            bsl = slice(b0, b0 + NT)

            # ---- loads -----------------------------------------------------
            xt = xp.tile([128, KC * NT], F32R, tag="xt")
            for k in range(KC):
                nc.sync.dma_start(out=xt[:, k * NT:(k + 1) * NT],
                                  in_=xt_d[k * 128:(k + 1) * 128, bsl])
            trt = xp.tile([1, NT], F32R, tag="trt")
            nc.sync.dma_start(out=trt[:], in_=xt_d[D:D + 1, bsl])
            xdt = xp.tile([128, KC * NT], F32R, tag="xdt")
            for k in range(KC):
                nc.sync.dma_start(out=xdt[:, k * NT:(k + 1) * NT],
                                  in_=xdt_d[k * 128:(k + 1) * 128, bsl])
            sz = xp.tile([1, NT], F32, tag="sz")
            nc.sync.dma_start(out=sz[:], in_=size_d[0:1, bsl])

            def xck(t, k):
                return t[:, k * NT:(k + 1) * NT]

            # ---- encoder layer 1 (K = 2049, chunked) -----------------------
            pre1 = pp.tile([128, NT], F32, tag="ps")
            for k in range(KC):
                nc.tensor.matmul(pre1[:], w1_sb[:, k * 128:(k + 1) * 128],
                                 xck(xt, k), start=(k == 0), stop=False)
            nc.tensor.matmul(pre1[:], w1r_sb[:], trt[:],
                             start=False, stop=True)
            a1 = ap_.tile([128, NT], F32R, tag="a1")
            nc.scalar.activation(a1[:], pre1[:], AF.Sigmoid, bias=eb1)

            dpre1 = pp.tile([128, NT], F32, tag="ps")
            for k in range(KC):
                nc.tensor.matmul(dpre1[:], w1_sb[:, k * 128:(k + 1) * 128],
                                 xck(xdt, k), start=(k == 0), stop=False)
            nc.tensor.matmul(dpre1[:], w1r_sb[:], trt[:],
                             start=False, stop=True)
            t1 = ap_.tile([128, NT], F32, tag="t1")
            nc.vector.scalar_tensor_tensor(t1[:], f(a1[:]), 1.0, f(a1[:]),
                                           OP.subtract, OP.mult)
            dz1 = ap_.tile([128, NT], F32R, tag="dz1")
            nc.vector.scalar_tensor_tensor(dz1[:], t1[:], -1.0, dpre1[:],
                                           OP.mult, OP.mult)

            # ---- encoder layers 2-4 ---------------------------------------
            pre2 = pp.tile([64, NT], F32, tag="ps")
            nc.tensor.matmul(pre2[:], w2, a1[:], start=True, stop=True)
            a2 = ap_.tile([64, NT], F32R, tag="a2")
            nc.scalar.activation(a2[:], pre2[:], AF.Sigmoid, bias=eb2)
            dpre2 = pp.tile([64, NT], F32, tag="ps")
            nc.tensor.matmul(dpre2[:], w2, dz1[:], start=True, stop=True)
            t2 = ap_.tile([64, NT], F32, tag="t2")
            nc.vector.scalar_tensor_tensor(t2[:], f(a2[:]), 1.0, f(a2[:]),
                                           OP.subtract, OP.mult)
            dz2 = ap_.tile([64, NT], F32R, tag="dz2")
            nc.vector.scalar_tensor_tensor(dz2[:], t2[:], -1.0, dpre2[:],
                                           OP.mult, OP.mult)

            pre3 = pp.tile([32, NT], F32, tag="ps")
            nc.tensor.matmul(pre3[:], w3, a2[:], start=True, stop=True)
            a3 = ap_.tile([32, NT], F32R, tag="a3")
            nc.scalar.activation(a3[:], pre3[:], AF.Sigmoid, bias=eb3)
            dpre3 = pp.tile([32, NT], F32, tag="ps")
            nc.tensor.matmul(dpre3[:], w3, dz2[:], start=True, stop=True)
            t3 = ap_.tile([32, NT], F32, tag="t3")
            nc.vector.scalar_tensor_tensor(t3[:], f(a3[:]), 1.0, f(a3[:]),
                                           OP.subtract, OP.mult)
            dz3 = ap_.tile([32, NT], F32R, tag="dz3")
            nc.vector.scalar_tensor_tensor(dz3[:], t3[:], -1.0, dpre3[:],
                                           OP.mult, OP.mult)

            z_ps = pp.tile([3, NT], F32, tag="ps")
            nc.tensor.matmul(z_ps[:], w4, a3[:], start=True, stop=True)
            z_sb = ap_.tile([3, NT], F32, tag="z")
            nc.scalar.activation(z_sb[:], z_ps[:], AF.Identity, bias=eb4)
            nc.sync.dma_start(out=z_d[:, bsl], in_=z_sb[:])
            zr = ap_.tile([3, NT], F32R, tag="zr")
            nc.scalar.activation(zr[:], z_ps[:], AF.Identity, bias=eb4)

            zdt_ps = pp.tile([3, NT], F32, tag="ps")
            nc.tensor.matmul(zdt_ps[:], w4, dz3[:], start=True, stop=True)

            # ---- SINDy library / z_dot_pred --------------------------------
            # hrep = [s, s, s] via selector matmul, then cumulative products.
            hrep_ps = pp.tile([3, NT], F32, tag="ps")
            nc.tensor.matmul(hrep_ps[:], selh, zr[:], start=True, stop=True)
            p3 = ap_.tile([3, NT], F32R, tag="p3")        # [s^2, sd, st]
            nc.vector.tensor_mul(p3[:], z_sb[:], hrep_ps[:])
            p3b = ap_.tile([3, NT], F32R, tag="p3b")      # [s^3, s^2 d, s^2 t]
            nc.vector.tensor_mul(p3b[:], f(p3[:]), hrep_ps[:])
            zdp_ps = pp.tile([3, NT], F32, tag="ps")
            nc.tensor.matmul(zdp_ps[:], cz, zr[:], start=True, stop=False)
            nc.tensor.matmul(zdp_ps[:], cpm, p3[:], start=False, stop=False)
            nc.tensor.matmul(zdp_ps[:], cq, p3b[:], start=False, stop=True)
            zdp = ap_.tile([3, NT], F32R, tag="zdp")
            nc.scalar.activation(zdp[:], zdp_ps[:], AF.Identity, bias=c0)

            # ---- per-tile loss partials -----------------------------------
            misc = cp_.tile([3, 3], F32, tag="misc")
            nc.vector.memset(misc[:], 0.0)

            # sindy_z: sum((z_dot_true - z_dot_pred)^2)
            dsz = ap_.tile([3, NT], F32, tag="dsz")
            nc.vector.tensor_sub(dsz[:], zdt_ps[:], f(zdp[:]))
            dszs = ap_.tile([3, NT], F32, tag="dszs")
            nc.scalar.activation(dszs[:], dsz[:], AF.Square,
                                 accum_out=misc[0:3, 2:3])

            # loss_po: sum((z0 - size)^2)
            dpo = ap_.tile([1, NT], F32, tag="dpo")
            nc.vector.tensor_sub(dpo[:], z_sb[0:1, :], sz[:])
            dpos = ap_.tile([1, NT], F32, tag="dpos")
            nc.scalar.activation(dpos[:], dpo[:], AF.Square,
                                 accum_out=misc[0:1, 0:1])

            # loss_tr: sum(softplus(l) - l * treatment), stable softplus
            lrep_ps = pp.tile([1, NT], F32, tag="ps")
            nc.tensor.matmul(lrep_ps[:], sell, zr[:], start=True, stop=True)
            al = ap_.tile([1, NT], F32, tag="al")
            nc.scalar.activation(al[:], lrep_ps[:], AF.Abs)
            ex = ap_.tile([1, NT], F32, tag="ex")
            nc.scalar.activation(ex[:], al[:], AF.Exp, scale=-1.0)
            l1p = ap_.tile([1, NT], F32, tag="l1p")
            nc.scalar.activation(l1p[:], ex[:], AF.Ln, bias=1.0)
            rl = ap_.tile([1, NT], F32, tag="rl")
            nc.scalar.activation(rl[:], lrep_ps[:], AF.Relu)
            lt = ap_.tile([1, NT], F32, tag="lt")
            nc.vector.tensor_mul(lt[:], lrep_ps[:], f(trt[:]))
            sfu = ap_.tile([1, NT], F32, tag="sfu")
            nc.vector.tensor_add(sfu[:], rl[:], l1p[:])
            sfw = ap_.tile([1, NT], F32, tag="sfw")
            nc.vector.scalar_tensor_tensor(sfw[:], sfu[:], 1.0, lt[:],
                                           OP.mult, OP.subtract,
                                           accum_out=misc[0:1, 1:2])

            # ---- decoder forward + derivative chain ------------------------
            h1_ps = pp.tile([32, NT], F32, tag="ps")
            nc.tensor.matmul(h1_ps[:], v1, zr[:], start=True, stop=True)
            h1 = ap_.tile([32, NT], F32R, tag="h1")
            nc.scalar.activation(h1[:], h1_ps[:], AF.Sigmoid, bias=db1)
            dd1_ps = pp.tile([32, NT], F32, tag="ps")
            nc.tensor.matmul(dd1_ps[:], v1, zdp[:], start=True, stop=True)
            u1 = ap_.tile([32, NT], F32, tag="u1")
            nc.vector.scalar_tensor_tensor(u1[:], f(h1[:]), 1.0, f(h1[:]),
                                           OP.subtract, OP.mult)
            dd1 = ap_.tile([32, NT], F32R, tag="dd1")
            nc.vector.scalar_tensor_tensor(dd1[:], u1[:], -1.0, dd1_ps[:],
                                           OP.mult, OP.mult)

            h2_ps = pp.tile([64, NT], F32, tag="ps")
            nc.tensor.matmul(h2_ps[:], v2, h1[:], start=True, stop=True)
            h2 = ap_.tile([64, NT], F32R, tag="h2")
            nc.scalar.activation(h2[:], h2_ps[:], AF.Sigmoid, bias=db2)
            dd2_ps = pp.tile([64, NT], F32, tag="ps")
            nc.tensor.matmul(dd2_ps[:], v2, dd1[:], start=True, stop=True)
            u2 = ap_.tile([64, NT], F32, tag="u2")
            nc.vector.scalar_tensor_tensor(u2[:], f(h2[:]), 1.0, f(h2[:]),
                                           OP.subtract, OP.mult)
            dd2 = ap_.tile([64, NT], F32R, tag="dd2")
            nc.vector.scalar_tensor_tensor(dd2[:], u2[:], -1.0, dd2_ps[:],
                                           OP.mult, OP.mult)

            h3_ps = pp.tile([128, NT], F32, tag="ps")
            nc.tensor.matmul(h3_ps[:], v3, h2[:], start=True, stop=True)
            h3 = ap_.tile([128, NT], F32R, tag="h3")
            nc.scalar.activation(h3[:], h3_ps[:], AF.Sigmoid, bias=db3)
            dd3_ps = pp.tile([128, NT], F32, tag="ps")
            nc.tensor.matmul(dd3_ps[:], v3, dd2[:], start=True, stop=True)
            u3 = ap_.tile([128, NT], F32, tag="u3")
            nc.vector.scalar_tensor_tensor(u3[:], f(h3[:]), 1.0, f(h3[:]),
                                           OP.subtract, OP.mult)
            dd3 = ap_.tile([128, NT], F32R, tag="dd3")
            nc.vector.scalar_tensor_tensor(dd3[:], u3[:], -1.0, dd3_ps[:],
                                           OP.mult, OP.mult)

            # ---- decoder layer 4, chunked over the 2048 output features ----
            racc = cp_.tile([128, MC], F32, tag="racc")
            sacc = cp_.tile([128, MC], F32, tag="sacc")
            for m in range(MC):
                v4c = v4_sb[:, m * 128:(m + 1) * 128]
                db4c = db4m[:, m:m + 1]

                xh_ps = pp.tile([128, NT], F32, tag="ps")
                nc.tensor.matmul(xh_ps[:], v4c, h3[:], start=True, stop=True)
                xh_sb = sp.tile([128, NT], F32, tag="xh")
                nc.scalar.activation(xh_sb[:], xh_ps[:], AF.Identity,
                                     bias=db4c)
                nc.sync.dma_start(out=xh_d[m * 128:(m + 1) * 128, bsl],
                                  in_=xh_sb[:])
                dif = sp.tile([128, NT], F32, tag="dif")
                nc.vector.scalar_tensor_tensor(dif[:], xh_ps[:], db4c,
                                               f(xck(xt, m)), OP.add,
                                               OP.subtract)
                difs = sp.tile([128, NT], F32, tag="difs")
                nc.scalar.activation(difs[:], dif[:], AF.Square,
                                     accum_out=racc[:, m:m + 1])

                xdp_ps = pp.tile([128, NT], F32, tag="ps")
                nc.tensor.matmul(xdp_ps[:], v4c, dd3[:], start=True,
                                 stop=True)
                dif2 = sp.tile([128, NT], F32, tag="dif2")
                nc.vector.tensor_sub(dif2[:], xdp_ps[:], f(xck(xdt, m)))
                dif2s = sp.tile([128, NT], F32, tag="dif2s")
                nc.vector.scalar_tensor_tensor(dif2s[:], dif2[:], 1.0,
                                               dif2[:], OP.mult, OP.mult,
                                               accum_out=sacc[:, m:m + 1])

            # ---- partials out ---------------------------------------------
            pc = bt * PBT
            nc.sync.dma_start(out=pr_d[:, pc:pc + 16], in_=racc[:])
            nc.sync.dma_start(out=pr_d[:, pc + 16:pc + 32], in_=sacc[:])
            nc.sync.dma_start(out=pr_d[0:3, pc + 32:pc + 35], in_=misc[:])

    nc.compile()
    _BUILt = nc
    return nc


def prep_inputs(x, x_dot, treatment, size,
                eW1, eb1, eW2, eb2, eW3, eb3, eW4, eb4,
                dW1, db1, dW2, db2, dW3, db3, dW4, db4, coefficients):
    """Host-side shard + layout prep. Returns per-core input maps."""
    f32 = np.float32
    xtT = np.empty((D + 1, B), f32)
    xtT[:D] = x.T
    xtT[D] = treatment[:, 0]
    xdtT = np.empty((D + 1, B), f32)
    xdtT[:D] = x_dot.T
    xdtT[D] = treatment[:, 0]
    sizeT = np.ascontiguousarray(size.T)

    w1T = np.ascontiguousarray(eW1.T, f32)
    v4T = np.ascontiguousarray(dW4.T, f32)

    sw = np.zeros((128, SW_COLS), f32)

    def put(spec, val):
        p, c0, w = spec
        assert val.shape == (p, w), (spec, val.shape)
        sw[0:p, c0:c0 + w] = val

    C = np.asarray(coefficients, f32)
    selh = np.zeros((3, 3), f32)
    selh[0, :] = 1.0
    sell = np.zeros((3, 1), f32)
    sell[2, 0] = 1.0
    cz = np.zeros((3, 3), f32)
    cz[0, :] = C[1, :]
    cq = np.zeros((3, 3), f32)
    cq[1, :] = C[5, :]
    cq[2, :] = C[6, :]

    put(SW_W2, eW2.T)
    put(SW_W3, eW3.T)
    put(SW_W4, eW4.T)
    put(SW_V1, dW1.T)
    put(SW_V2, dW2.T)
    put(SW_V3, dW3.T)
    put(SW_DB4, np.ascontiguousarray(db4.reshape(MC, 128).T))
    put(SW_EB1, eb1.reshape(-1, 1))
    put(SW_EB2, eb2.reshape(-1, 1))
    put(SW_EB3, eb3.reshape(-1, 1))
    put(SW_EB4, eb4.reshape(-1, 1))
    put(SW_DB1, db1.reshape(-1, 1))
    put(SW_DB2, db2.reshape(-1, 1))
    put(SW_DB3, db3.reshape(-1, 1))
    put(SW_SELH, selh)
    put(SW_SELL, sell)
    put(SW_CZ, cz)
    put(SW_CP, np.ascontiguousarray(C[2:5, :]))
    put(SW_CQ, cq)
    put(SW_C0, C[0, :].reshape(3, 1))

    in_maps = []
    for c in range(NC_CORES):
        sl = slice(c * BS, (c + 1) * BS)
        in_maps.append({
            "xt": np.ascontiguousarray(xtT[:, sl]),
            "xdt": np.ascontiguousarray(xdtT[:, sl]),
            "size": np.ascontiguousarray(sizeT[:, sl]),
            "w1T": w1T,
            "v4T": v4T,
            "smallw": sw,
        })
    return in_maps


def finish(results, coefficients):
    """Gather per-core outputs into the reference's output pytree."""
    f32 = np.float32
    z = np.empty((B, 3), f32)
    x_hat = np.empty((B, D), f32)
    po = tr = rec = sx = szz = 0.0
    for c in range(NC_CORES):
        sl = slice(c * BS, (c + 1) * BS)
        r = results[c]
        z[sl] = r["zT"].T
        x_hat[sl] = r["xhatT"].T
        P = r["partials"].reshape(128, NBT, PBT).astype(np.float64)
        rec += P[:, :, 0:16].sum()
        sx += P[:, :, 16:32].sum()
        po += P[0, :, 32].sum()
        tr += P[0, :, 33].sum()
        szz += P[0:3, :, 34].sum()
    loss_po = np.float32(po / B)
    loss_tr = np.float32(tr / B)
    recon = np.float32(rec / (B * D))
    sindy_x = np.float32(sx / (B * D))
    sindy_z = np.float32(szz / (B * 3))
    l1 = np.float32(np.mean(np.abs(np.asarray(coefficients, np.float64))))
    return (z, x_hat, loss_po, loss_tr, recon, sindy_x, sindy_z, l1)


def kernel(**inputs):
    nc = build()
    in_maps = prep_inputs(**inputs)
    res = run_bass_kernel_spmd(nc, in_maps, list(range(NC_CORES)))
    return finish(res.results, inputs["coefficients"])
